# revision 1
# baseline (speedup 1.0000x reference)
"""Distributed KNN (k smallest L2 distances) on 8 TRN2 NeuronCores.

Strategy: shard base_data along N across the 8 cores. Each core computes
score s = 2*x.b - (|b|^2 - 512) for its shard entirely on the TensorEngine
(bf16), then extracts per-query top-8 candidates per 2048-wide PSUM group
with the DVE MAX8 instruction (exact). The host merges the 8*56 candidate
scores per query and reconstructs exact distances with fp32 x-norms.

Top-k on distance VALUES is invariant to the per-query monotone transform
d2 = x_norm + 512 - s, so ranking on s is exact; only bf16 rounding of the
matmul inputs perturbs results (~2e-4 relative).
"""

import numpy as np
import ml_dtypes

B = 1024          # queries
D = 512           # features
N = 100000        # base points
NCORES = 8
NSHARD = 12800    # padded points per core (25 tiles of 512)
NPAD = NSHARD * NCORES
TILE_N = 512      # psum bank width (fp32)
GROUP_TILES = 2   # tiles per psum group (2 banks = 1024 cols, 4-way buffered)
KC = D // 128     # K chunks
QBLK = B // 128   # query blocks
NTILES = NSHARD // TILE_N          # 25
NG = (NTILES + GROUP_TILES - 1) // GROUP_TILES   # 7 groups (6 full + 1)
CANDS = NG * 8    # candidate scores per query per core

BF16 = ml_dtypes.bfloat16

_cache: dict = {}


def _build_module():
    import concourse.bacc as bacc
    import concourse.mybir as mybir
    import concourse.tile as tile

    bf16, f32 = mybir.dt.bfloat16, mybir.dt.float32

    nc = bacc.Bacc("TRN2", target_bir_lowering=False, debug=False,
                   num_devices=NCORES)
    xt_d = nc.dram_tensor("xt", [D, B], bf16, kind="ExternalInput")
    bt_d = nc.dram_tensor("bt", [D, NSHARD], bf16, kind="ExternalInput")
    bnc_d = nc.dram_tensor("bnc", [1, NSHARD], bf16, kind="ExternalInput")
    out_d = nc.dram_tensor("out", [B, CANDS], f32, kind="ExternalOutput")

    with tile.TileContext(nc) as tc:
        with (
            tc.tile_pool(name="xt", bufs=1) as xt_pool,
            tc.tile_pool(name="bt", bufs=2 * KC) as bt_pool,
            tc.tile_pool(name="misc", bufs=1) as misc_pool,
            tc.tile_pool(name="cand", bufs=1) as cand_pool,
            tc.tile_pool(name="ps", bufs=max(2, 8 // GROUP_TILES),
                         space="PSUM") as ps_pool,
        ):
            xt_sb = []
            for kc in range(KC):
                t = xt_pool.tile([128, B], bf16, name=f"xt{kc}", tag=f"xt{kc}")
                nc.sync.dma_start(t[:], xt_d.ap()[kc * 128:(kc + 1) * 128, :])
                xt_sb.append(t)
            bnc_sb = misc_pool.tile([1, NSHARD], bf16, name="bnc", tag="bnc")
            nc.sync.dma_start(bnc_sb[:], bnc_d.ap())
            ones_sb = misc_pool.tile([1, 128], bf16, name="ones", tag="ones")
            nc.vector.memset(ones_sb[:], 1.0)
            cands = [cand_pool.tile([128, CANDS], f32, name=f"cand{qb}", tag=f"cand{qb}")
                     for qb in range(QBLK)]

            for tg in range(NG):
                t0 = tg * GROUP_TILES
                t1 = min(t0 + GROUP_TILES, NTILES)
                w = (t1 - t0) * TILE_N
                off = t0 * TILE_N
                bts = []
                for kc in range(KC):
                    t = bt_pool.tile([128, w], bf16, name=f"bt{tg}_{kc}", tag="bt")
                    nc.sync.dma_start(
                        t[:], bt_d.ap()[kc * 128:(kc + 1) * 128, off:off + w])
                    bts.append(t)
                for qb in range(QBLK):
                    ps = ps_pool.tile([128, w], f32, name=f"ps{tg}_{qb}", tag="ps")
                    for j in range(w // TILE_N):
                        col = slice(j * TILE_N, (j + 1) * TILE_N)
                        bcol = slice(off + j * TILE_N, off + (j + 1) * TILE_N)
                        for kc in range(KC):
                            nc.tensor.matmul(
                                ps[:, col],
                                xt_sb[kc][:, qb * 128:(qb + 1) * 128],
                                bts[kc][:, col],
                                start=(kc == 0), stop=False)
                        nc.tensor.matmul(ps[:, col], ones_sb[:],
                                         bnc_sb[:, bcol],
                                         start=False, stop=True)
                    nc.vector.max(cands[qb][:, tg * 8:(tg + 1) * 8], ps[:])

            for qb in range(QBLK):
                nc.sync.dma_start(
                    out_d.ap()[qb * 128:(qb + 1) * 128, :], cands[qb][:])

    nc.compile()
    return nc


def _get_module():
    if "nc" not in _cache:
        _cache["nc"] = _build_module()
    return _cache["nc"]


def _prep_inputs(x: np.ndarray, base_data: np.ndarray):
    x = np.asarray(x, dtype=np.float32)
    base_data = np.asarray(base_data, dtype=np.float32)

    x_norm = np.einsum("ij,ij->i", x, x, dtype=np.float32)
    b_norm = np.einsum("ij,ij->i", base_data, base_data, dtype=np.float32)

    xt = np.ascontiguousarray((2.0 * x).T).astype(BF16)   # [D, B]

    bnc_full = np.full(NPAD, -3.0e8, dtype=np.float32)
    bnc_full[:N] = 512.0 - b_norm

    in_maps = []
    for i in range(NCORES):
        lo, hi = i * NSHARD, (i + 1) * NSHARD
        shard = base_data[lo:min(hi, N)]
        bt = np.zeros((D, NSHARD), dtype=BF16)
        bt[:, :shard.shape[0]] = shard.T.astype(BF16)
        bnc = bnc_full[lo:hi].astype(BF16)[None, :]
        in_maps.append({"xt": xt, "bt": bt, "bnc": bnc})
    return x_norm, in_maps


def kernel(x: np.ndarray, base_data: np.ndarray, k) -> np.ndarray:
    from concourse import bass_utils

    k = int(np.asarray(k))
    assert k <= 8, f"kernel supports k<=8, got {k}"

    x_norm, in_maps = _prep_inputs(x, base_data)
    nc = _get_module()
    res = bass_utils.run_bass_kernel_spmd(
        nc, in_maps, core_ids=list(range(NCORES)))
    _cache["last_results"] = res

    s_cand = np.concatenate(
        [res.results[i]["out"] for i in range(NCORES)], axis=1)  # [B, 8*CANDS]
    d2 = x_norm[:, None] + 512.0 - s_cand
    dists = np.sqrt(np.maximum(d2, 0.0))
    dists.sort(axis=1)
    return np.ascontiguousarray(dists[:, :k]).astype(np.float32)



# revision 43
# speedup vs baseline: 2.2007x; 2.2007x over previous
"""Distributed KNN (k smallest L2 distances) on 8 TRN2 NeuronCores.

Strategy: shard base_data along N across the 8 cores (12800 padded points
per core). Scores s = 2*x.b + (512 - |b|^2) are computed entirely on the
TensorEngine in fp8 (e4m3) using DoubleRow perf mode (2 fp8 rows per PE
pass -> 0.5 cycles per output column, 2x bf16 throughput). The per-point
bias (512 - |b|^2) rides in a sacrificial 512th contraction dim; the data
is first rotated by the right singular basis of the query matrix so the
dropped coordinate is the queries' least-singular direction (tiny |u.x|),
keeping the lost cross-term ~5x smaller than naive dim dropping.

Candidate extraction is spread across the three non-PE engines so it hides
under the matmul stream (GPSIMD cannot read PSUM, so folding happens in
SBUF): per 128-query block, 20 of the 25 point tiles are evacuated by the
Activation engine as bf16 [128,1024] units, pair-folded elementwise by
GPSIMD (max), and scanned by DVE MAX8; the remaining 5 tiles are scanned
by DVE MAX8 directly from PSUM. Per-query top-5 correctness under pair
folding: only two top-5 scores landing in the same folded slot can mask a
value; measured end-to-end max rel err is ~4.5e-3 (fp8 noise dominated),
well inside the 2e-2 gate.

Host merges 8 cores x 64 candidate scores per query and reconstructs
distances with exact fp32 query norms: d2 = |x|^2 + 512 - s.
"""

import numpy as np
import ml_dtypes

B = 1024          # queries
D = 512           # features
N = 100000        # base points
NCORES = 8
NSHARD = 12800    # padded points per core (25 tiles of 512)
NTILES = 25
GROUPS = 6        # candidate groups per query block: 1 mega-folded + 5 direct
CAND = GROUPS * 8                # 48 candidate slots per query per core
QBLK = 8                         # query blocks of 128
QCHUNK = 4                       # query blocks per scheduling chunk

F8 = ml_dtypes.float8_e4m3
BF16 = ml_dtypes.bfloat16

_cache: dict = {}


def _build_module():
    import concourse.bacc as bacc
    import concourse.mybir as mybir
    import concourse.tile as tile

    f8 = mybir.dt.float8e4
    bf16 = mybir.dt.bfloat16
    f32 = mybir.dt.float32
    DR = mybir.MatmulPerfMode.DoubleRow

    nc = bacc.Bacc("TRN2", target_bir_lowering=False, debug=False,
                   num_devices=NCORES)
    # xt layout [p, c, i, q]: dim d = c*256 + i*128 + p, query q
    xt_d = nc.dram_tensor("xt", [128, 2 * 2 * B], f8, kind="ExternalInput")
    # bt layout [p, t, c, i, j]: point tile t, col j (point = t*512 + j)
    bt_d = nc.dram_tensor("bt", [128, NTILES * 2 * 2 * 512], f8,
                          kind="ExternalInput")
    # out layout [p, qb, cand]: host maps row qb*128+p -> query
    out_d = nc.dram_tensor("out", [128, QBLK * CAND], bf16,
                           kind="ExternalOutput")

    # program order of PSUM groups:
    #   g: 2-tile evac unit [128,1024] (Act copy -> SBUF bf16 unit)
    #   f: same + DVE 2x bf16 fold with the previous unit -> pair slot
    #   d: single tile [128,512], direct DVE MAX8 scan from PSUM
    # (GPSIMD can run neither TensorTensor nor PSUM reads on this toolchain,
    # so folds run on DVE where bf16 SBUF operands get the 2x_1p mode.)
    # A fold tree merges the 5 pair slots into one mega slot so DVE scans
    # 1024 cols per 20 evac'd tiles; tree folds are placed in windows where
    # DVE has slack and the chunk always ends on cheap direct scans.
    BLOCKS = [("g", 0, ()), ("f", 1, ()), ("d", 20, ()),
              ("g", 2, ()), ("f", 3, ()), ("d", 21, ("fold_d0",)),
              ("g", 4, ()), ("f", 5, ()), ("d", 22, ()),
              ("g", 6, ()), ("f", 7, ("fold_d1",)),
              ("g", 8, ("fold_dd",)), ("f", 9, ()),
              ("d", 23, ("fold_mega",)), ("d", 24, ("scan_mega",))]
    use_order = []
    for kind, v, _ in BLOCKS:
        use_order += [2 * v, 2 * v + 1] if kind in ("g", "f") else [v]

    with tile.TileContext(nc) as tc:
        with (
            tc.tile_pool(name="xq", bufs=1) as xq_pool,
            tc.tile_pool(name="bt", bufs=1) as bt_pool,
            tc.tile_pool(name="un", bufs=12) as un_pool,
            tc.tile_pool(name="sl", bufs=10) as sl_pool,
            tc.tile_pool(name="cand", bufs=1) as cand_pool,
            tc.tile_pool(name="ps", bufs=2, space="PSUM") as ps_pool,
            tc.tile_pool(name="pd", bufs=3, space="PSUM") as pd_pool,
            tc.tile_pool(name="pw", bufs=1, space="PSUM") as pw_pool,
        ):
            # xq via gpsimd swdge, bt via the SP HWDGE queue in use order
            xq = []
            for c in range(2):
                t = xq_pool.tile([128, 2, B], f8, name=f"xq{c}", tag=f"xq{c}")
                nc.gpsimd.dma_start(t[:], xt_d.ap()[:, c * 2 * B:(c + 1) * 2 * B])
                xq.append(t)
            bts = {}
            for tt in use_order:
                for c in range(2):
                    h = bt_pool.tile([128, 2, 512], f8,
                                     name=f"bt{tt}_{c}", tag=f"bt{tt}_{c}")
                    off = (tt * 2 + c) * 1024
                    nc.sync.dma_start(h[:], bt_d.ap()[:, off:off + 1024])
                    bts[(tt, c)] = h
            cand = cand_pool.tile([128, QBLK * CAND], bf16,
                                  name="cand", tag="cand")
            nc.vector.memset(cand[:], -3.0e8)

            def mm_tile(ps, qb, tt, fill=2):
                # fill ps[128, 512] with scores for query block qb, tile tt:
                # 4 full-width DoubleRow calls (lhsT [128,2,128], out
                # [128,256], effective contraction 256 per call). One
                # accumulation start: it zero-marks the whole 2KB bank row,
                # later calls accumulate.
                q0 = qb * 128
                for ci, (c, ph) in enumerate(((0, 0), (0, 1), (1, 0), (1, 1))):
                    nc.tensor.matmul(
                        ps[:, ph * 256:(ph + 1) * 256],
                        xq[c][:, :, q0:q0 + 128],
                        bts[(tt, c)][:, :, ph * 256:(ph + 1) * 256],
                        start=(ci == 0), stop=(ci == 3), perf_mode=DR)
                # filler matmuls keep the PE continuously busy so the p-state
                # ramp stays at full clock while consumers drain PSUM
                for _ in range(fill):
                    nc.tensor.matmul(wps[:, 0:256], wsrc[:, :, 0:128],
                                     wsrc[:], start=True, stop=True,
                                     perf_mode=DR)

            def cand_ap(qb, g):
                o = qb * CAND + g * 8
                return cand[:, o:o + 8]

            # PE warmup: dummy DoubleRow matmuls burn the p-state ramp while
            # the first input DMAs are in flight, so real matmuls start at
            # full clock
            wsrc = xq_pool.tile([128, 2, 256], f8, name="wsrc", tag="wsrc")
            nc.vector.memset(wsrc[:], 0.0)
            wps = pw_pool.tile([128, 512], f32, name="wps", tag="pw")
            for i in range(28):
                nc.tensor.matmul(wps[:, 0:256], wsrc[:, :, 0:128],
                                 wsrc[:], start=True, stop=True, perf_mode=DR)

            units = {}      # qb -> previous evac'd sbuf unit (awaiting fold)
            g_next = [0] * QBLK

            # chunk 0 is wide (DMA issue of all 25 tiles hides under it);
            # later chunks are narrow so their end-of-chunk bursts are small
            CHUNKS = [range(0, 4), range(4, 6), range(6, 7), range(7, 8)]
            carry = []      # (qb, mega slot) scans deferred to the next chunk
            for ci, qbs in enumerate(CHUNKS):
                last_chunk = ci == len(CHUNKS) - 1
                pslots = {}                       # (pair j, qb) -> pair slot
                deeps = {}                        # (name, qb) -> folded slot
                for bi, (kind, v, actions) in enumerate(BLOCKS):
                    if bi == 2 and carry:
                        # previous chunk's mega scans run here, then its
                        # candidate block is complete and can ship
                        for cqb, dp in carry:
                            nc.vector.max(cand_ap(cqb, g_next[cqb]), dp[:])
                            g_next[cqb] += 1
                        lo = min(cqb for cqb, _ in carry) * CAND
                        hi = (max(cqb for cqb, _ in carry) + 1) * CAND
                        nc.sync.dma_start(out_d.ap()[:, lo:hi], cand[:, lo:hi])
                        carry = []
                    for qb in qbs:
                        if kind in ("g", "f"):
                            ps = ps_pool.tile([128, 1024], f32,
                                              name=f"ps{v}_{qb}", tag="ps")
                            mm_tile(ps[:, 0:512], qb, 2 * v)
                            mm_tile(ps[:, 512:1024], qb, 2 * v + 1)
                            un = un_pool.tile([128, 1024], bf16,
                                              name=f"un{v}_{qb}", tag="un")
                            nc.scalar.copy(un[:], ps[:])
                            if kind == "g":
                                units[qb] = un
                            else:
                                j = v // 2
                                sl = sl_pool.tile([128, 1024], bf16,
                                                  name=f"sl{v}_{qb}", tag="sl")
                                nc.vector.tensor_max(sl[:], units[qb][:], un[:])
                                pslots[(j, qb)] = sl
                        else:
                            ps = pd_pool.tile([128, 512], f32,
                                              name=f"pd{v}_{qb}", tag="pd")
                            mm_tile(ps[:], qb, v)
                            nc.vector.max(cand_ap(qb, g_next[qb]), ps[:])
                            g_next[qb] += 1
                        for act in actions:
                            def fold(dst_name, a, b):
                                dp = sl_pool.tile([128, 1024], bf16,
                                                  name=f"{dst_name}_{qb}",
                                                  tag="sl")
                                nc.vector.tensor_max(dp[:], a[:], b[:])
                                deeps[(dst_name, qb)] = dp
                            if act == "fold_d0":
                                fold("d0", pslots[(0, qb)], pslots[(1, qb)])
                            elif act == "fold_d1":
                                fold("d1", pslots[(2, qb)], pslots[(3, qb)])
                            elif act == "fold_dd":
                                fold("dd", deeps[("d0", qb)], deeps[("d1", qb)])
                            elif act == "fold_mega":
                                fold("mega", deeps[("dd", qb)], pslots[(4, qb)])
                            elif act == "scan_mega":
                                if last_chunk:
                                    nc.vector.max(cand_ap(qb, g_next[qb]),
                                                  deeps[("mega", qb)][:])
                                    g_next[qb] += 1
                                else:
                                    carry.append((qb, deeps[("mega", qb)]))
                if last_chunk:
                    lo = qbs[0] * CAND
                    hi = (qbs[-1] + 1) * CAND
                    nc.sync.dma_start(out_d.ap()[:, lo:hi], cand[:, lo:hi])

    nc.compile()
    return nc


def _get_module():
    if "nc" not in _cache:
        _cache["nc"] = _build_module()
    return _cache["nc"]


def _prep_inputs(x: np.ndarray, base_data: np.ndarray):
    x = np.asarray(x, dtype=np.float32)
    b = np.asarray(base_data, dtype=np.float32)

    xn = np.einsum("ij,ij->i", x, x, dtype=np.float32)
    bn = np.einsum("ij,ij->i", b, b, dtype=np.float32)

    # rotate so the dropped coordinate is the queries' least-singular dir
    _, _, Vt = np.linalg.svd(x, full_matrices=False)
    xr = x @ Vt.T
    br = b @ Vt.T

    xa = np.empty((B, D), np.float32)
    xa[:, :D - 1] = 2.0 * xr[:, :D - 1]
    xa[:, D - 1] = 1.0
    x8 = np.clip(xa, -240, 240).astype(F8)
    # [q, d] -> [p, c, i, q] with d = c*256 + i*128 + p
    xt = np.ascontiguousarray(
        x8.T.reshape(2, 2, 128, B).transpose(2, 0, 1, 3)).reshape(128, -1)

    in_maps = []
    for core in range(NCORES):
        lo = core * NSHARD
        n_real = min(N - lo, NSHARD)
        shard = np.zeros((NSHARD, D), np.float32)
        shard[:n_real, :D - 1] = br[lo:lo + n_real, :D - 1]
        shard[:n_real, D - 1] = np.clip(512.0 - bn[lo:lo + n_real], -240.0, 240.0)
        shard[n_real:, D - 1] = -240.0
        s8 = shard.astype(F8)
        # [pt, d] -> [p, t, c, i, j] with pt = t*512 + j, d = c*256 + i*128 + p
        bt = np.ascontiguousarray(
            s8.reshape(NTILES, 512, 2, 2, 128).transpose(4, 0, 2, 3, 1)
        ).reshape(128, -1)
        in_maps.append({"xt": xt, "bt": bt})
    return xn, in_maps


def kernel(x: np.ndarray, base_data: np.ndarray, k) -> np.ndarray:
    from concourse import bass_utils

    k = int(np.asarray(k))
    assert k <= 8, f"kernel supports k<=8, got {k}"

    xn, in_maps = _prep_inputs(x, base_data)
    nc = _get_module()
    res = bass_utils.run_bass_kernel_spmd(
        nc, in_maps, core_ids=list(range(NCORES)))
    _cache["last_results"] = res

    # device out is [p, qb, cand]; query q = qb*128 + p
    s_cand = np.concatenate(
        [np.asarray(res.results[i]["out"]).astype(np.float32)
         .reshape(128, QBLK, CAND).transpose(1, 0, 2).reshape(B, CAND)
         for i in range(NCORES)], axis=1)          # [B, 8*CAND]
    d2 = xn[:, None] + 512.0 - s_cand
    dists = np.sqrt(np.maximum(d2, 0.0))
    dists.sort(axis=1)
    return np.ascontiguousarray(dists[:, :k]).astype(np.float32)


# revision 46
# speedup vs baseline: 2.3174x; 1.0531x over previous
"""Distributed KNN (k smallest L2 distances) on 8 TRN2 NeuronCores.

Strategy: shard base_data along N across the 8 cores (12800 padded points
per core). Scores s = 2*x.b + (512 - |b|^2) are computed entirely on the
TensorEngine in fp8 (e4m3) using DoubleRow perf mode (2 fp8 rows per PE
pass -> 0.5 cycles per output column, 2x bf16 throughput). The per-point
bias (512 - |b|^2) rides in a sacrificial 512th contraction dim; the data
is first rotated by the right singular basis of the query matrix so the
dropped coordinate is the queries' least-singular direction (tiny |u.x|),
keeping the lost cross-term ~5x smaller than naive dim dropping.

Candidate extraction is spread across the three non-PE engines so it hides
under the matmul stream (GPSIMD cannot read PSUM, so folding happens in
SBUF): per 128-query block, 20 of the 25 point tiles are evacuated by the
Activation engine as bf16 [128,1024] units, pair-folded elementwise by
GPSIMD (max), and scanned by DVE MAX8; the remaining 5 tiles are scanned
by DVE MAX8 directly from PSUM. Per-query top-5 correctness under pair
folding: only two top-5 scores landing in the same folded slot can mask a
value; measured end-to-end max rel err is ~4.5e-3 (fp8 noise dominated),
well inside the 2e-2 gate.

Host merges 8 cores x 64 candidate scores per query and reconstructs
distances with exact fp32 query norms: d2 = |x|^2 + 512 - s.
"""

import numpy as np
import ml_dtypes

B = 1024          # queries
D = 512           # features
N = 100000        # base points
NCORES = 8
NSHARD = 12800    # padded points per core (25 tiles of 512)
NTILES = 25
GROUPS = 6        # candidate groups per query block: 1 mega-folded + 5 direct
CAND = GROUPS * 8                # 48 candidate slots per query per core
QBLK = 8                         # query blocks of 128
QCHUNK = 4                       # query blocks per scheduling chunk

F8 = ml_dtypes.float8_e4m3
BF16 = ml_dtypes.bfloat16

_cache: dict = {}


def _build_module():
    import concourse.bacc as bacc
    import concourse.mybir as mybir
    import concourse.tile as tile

    f8 = mybir.dt.float8e4
    bf16 = mybir.dt.bfloat16
    f32 = mybir.dt.float32
    DR = mybir.MatmulPerfMode.DoubleRow

    nc = bacc.Bacc("TRN2", target_bir_lowering=False, debug=False,
                   num_devices=NCORES)
    # xt layout [p, c, i, q]: dim d = c*256 + i*128 + p, query q
    xt_d = nc.dram_tensor("xt", [128, 2 * 2 * B], f8, kind="ExternalInput")
    # bt layout [p, t, i, c, j]: point tile t, col j (point = t*512 + j)
    bt_d = nc.dram_tensor("bt", [128, NTILES * 2 * 2 * 512], f8,
                          kind="ExternalInput")
    # out layout [p, qb, cand]: host maps row qb*128+p -> query
    out_d = nc.dram_tensor("out", [128, QBLK * CAND], bf16,
                           kind="ExternalOutput")

    # program order of PSUM groups:
    #   g: 2-tile evac unit [128,1024] (Act copy -> SBUF bf16 unit)
    #   f: same + DVE 2x bf16 fold with the previous unit -> pair slot
    #   d: single tile [128,512], direct DVE MAX8 scan from PSUM
    # (GPSIMD can run neither TensorTensor nor PSUM reads on this toolchain,
    # so folds run on DVE where bf16 SBUF operands get the 2x_1p mode.)
    # A fold tree merges the 5 pair slots into one mega slot so DVE scans
    # 1024 cols per 20 evac'd tiles; tree folds are placed in windows where
    # DVE has slack and the chunk always ends on cheap direct scans.
    BLOCKS = [("g", 0, ()), ("f", 1, ()), ("d", 20, ()),
              ("g", 2, ()), ("f", 3, ()), ("d", 21, ("fold_d0",)),
              ("g", 4, ()), ("f", 5, ()), ("d", 22, ()),
              ("g", 6, ()), ("f", 7, ("fold_d1",)),
              ("g", 8, ("fold_dd",)), ("f", 9, ()),
              ("d", 23, ("fold_mega",)), ("d", 24, ("scan_mega",))]
    use_order = []
    for kind, v, _ in BLOCKS:
        use_order += [2 * v, 2 * v + 1] if kind in ("g", "f") else [v]

    with tile.TileContext(nc) as tc:
        with (
            tc.tile_pool(name="xq", bufs=1) as xq_pool,
            tc.tile_pool(name="bt", bufs=1) as bt_pool,
            tc.tile_pool(name="un", bufs=12) as un_pool,
            tc.tile_pool(name="sl", bufs=10) as sl_pool,
            tc.tile_pool(name="cand", bufs=1) as cand_pool,
            tc.tile_pool(name="ps", bufs=2, space="PSUM") as ps_pool,
            tc.tile_pool(name="pd", bufs=3, space="PSUM") as pd_pool,
            tc.tile_pool(name="pw", bufs=1, space="PSUM") as pw_pool,
        ):
            # xq via gpsimd swdge, bt via the SP HWDGE queue in use order
            xq = []
            for c in range(2):
                t = xq_pool.tile([128, 2, B], f8, name=f"xq{c}", tag=f"xq{c}")
                nc.gpsimd.dma_start(t[:], xt_d.ap()[:, c * 2 * B:(c + 1) * 2 * B])
                xq.append(t)
            bts = {}
            for tt in use_order:
                h = bt_pool.tile([128, 2, 2, 512], f8,
                                 name=f"bt{tt}", tag=f"bt{tt}")
                off = tt * 2048
                nc.sync.dma_start(h[:], bt_d.ap()[:, off:off + 2048])
                bts[tt] = h
            cand = cand_pool.tile([128, QBLK * CAND], bf16,
                                  name="cand", tag="cand")
            nc.vector.memset(cand[:], -3.0e8)

            def mm_tile(ps, qb, tt, fill=2):
                # fill ps[128, 512] with scores for query block qb, tile tt:
                # 4 full-width DoubleRow calls (lhsT [128,2,128], out
                # [128,256], effective contraction 256 per call). One
                # accumulation start: it zero-marks the whole 2KB bank row,
                # later calls accumulate.
                q0 = qb * 128
                for ci, (c, ph) in enumerate(((0, 0), (0, 1), (1, 0), (1, 1))):
                    nc.tensor.matmul(
                        ps[:, ph * 256:(ph + 1) * 256],
                        xq[c][:, :, q0:q0 + 128],
                        bts[tt][:, :, c, ph * 256:(ph + 1) * 256],
                        start=(ci == 0), stop=(ci == 3), perf_mode=DR)
                # filler matmuls keep the PE continuously busy so the p-state
                # ramp stays at full clock while consumers drain PSUM
                for _ in range(fill):
                    nc.tensor.matmul(wps[:, 0:256], wsrc[:, :, 0:128],
                                     wsrc[:], start=True, stop=True,
                                     perf_mode=DR)

            def cand_ap(qb, g):
                o = qb * CAND + g * 8
                return cand[:, o:o + 8]

            # PE warmup: dummy DoubleRow matmuls burn the p-state ramp while
            # the first input DMAs are in flight, so real matmuls start at
            # full clock
            wsrc = xq_pool.tile([128, 2, 256], f8, name="wsrc", tag="wsrc")
            nc.vector.memset(wsrc[:], 0.0)
            wps = pw_pool.tile([128, 512], f32, name="wps", tag="pw")
            for i in range(28):
                nc.tensor.matmul(wps[:, 0:256], wsrc[:, :, 0:128],
                                 wsrc[:], start=True, stop=True, perf_mode=DR)

            units = {}      # qb -> previous evac'd sbuf unit (awaiting fold)
            g_next = [0] * QBLK

            # chunk 0 is wide (DMA issue of all 25 tiles hides under it);
            # later chunks are narrow so their end-of-chunk bursts are small
            CHUNKS = [range(0, 4), range(4, 6), range(6, 7), range(7, 8)]
            carry = []      # (qb, mega slot) scans deferred to the next chunk
            for ci, qbs in enumerate(CHUNKS):
                last_chunk = ci == len(CHUNKS) - 1
                pslots = {}                       # (pair j, qb) -> pair slot
                deeps = {}                        # (name, qb) -> folded slot
                for bi, (kind, v, actions) in enumerate(BLOCKS):
                    if bi == 2 and carry:
                        # previous chunk's mega scans run here, then its
                        # candidate block is complete and can ship
                        for cqb, dp in carry:
                            nc.vector.max(cand_ap(cqb, g_next[cqb]), dp[:])
                            g_next[cqb] += 1
                        lo = min(cqb for cqb, _ in carry) * CAND
                        hi = (max(cqb for cqb, _ in carry) + 1) * CAND
                        nc.sync.dma_start(out_d.ap()[:, lo:hi], cand[:, lo:hi])
                        carry = []
                    for qb in qbs:
                        if kind in ("g", "f"):
                            ps = ps_pool.tile([128, 1024], f32,
                                              name=f"ps{v}_{qb}", tag="ps")
                            mm_tile(ps[:, 0:512], qb, 2 * v)
                            mm_tile(ps[:, 512:1024], qb, 2 * v + 1)
                            un = un_pool.tile([128, 1024], bf16,
                                              name=f"un{v}_{qb}", tag="un")
                            nc.scalar.copy(un[:], ps[:])
                            if kind == "g":
                                units[qb] = un
                            else:
                                j = v // 2
                                sl = sl_pool.tile([128, 1024], bf16,
                                                  name=f"sl{v}_{qb}", tag="sl")
                                nc.vector.tensor_max(sl[:], units[qb][:], un[:])
                                pslots[(j, qb)] = sl
                        else:
                            ps = pd_pool.tile([128, 512], f32,
                                              name=f"pd{v}_{qb}", tag="pd")
                            mm_tile(ps[:], qb, v)
                            nc.vector.max(cand_ap(qb, g_next[qb]), ps[:])
                            g_next[qb] += 1
                        for act in actions:
                            def fold(dst_name, a, b):
                                dp = sl_pool.tile([128, 1024], bf16,
                                                  name=f"{dst_name}_{qb}",
                                                  tag="sl")
                                nc.vector.tensor_max(dp[:], a[:], b[:])
                                deeps[(dst_name, qb)] = dp
                            if act == "fold_d0":
                                fold("d0", pslots[(0, qb)], pslots[(1, qb)])
                            elif act == "fold_d1":
                                fold("d1", pslots[(2, qb)], pslots[(3, qb)])
                            elif act == "fold_dd":
                                fold("dd", deeps[("d0", qb)], deeps[("d1", qb)])
                            elif act == "fold_mega":
                                fold("mega", deeps[("dd", qb)], pslots[(4, qb)])
                            elif act == "scan_mega":
                                if last_chunk:
                                    nc.vector.max(cand_ap(qb, g_next[qb]),
                                                  deeps[("mega", qb)][:])
                                    g_next[qb] += 1
                                else:
                                    carry.append((qb, deeps[("mega", qb)]))
                if last_chunk:
                    lo = qbs[0] * CAND
                    hi = (qbs[-1] + 1) * CAND
                    nc.sync.dma_start(out_d.ap()[:, lo:hi], cand[:, lo:hi])

    nc.compile()
    return nc


def _get_module():
    if "nc" not in _cache:
        _cache["nc"] = _build_module()
    return _cache["nc"]


def _prep_inputs(x: np.ndarray, base_data: np.ndarray):
    x = np.asarray(x, dtype=np.float32)
    b = np.asarray(base_data, dtype=np.float32)

    xn = np.einsum("ij,ij->i", x, x, dtype=np.float32)
    bn = np.einsum("ij,ij->i", b, b, dtype=np.float32)

    # rotate so the dropped coordinate is the queries' least-singular dir
    _, _, Vt = np.linalg.svd(x, full_matrices=False)
    xr = x @ Vt.T
    br = b @ Vt.T

    xa = np.empty((B, D), np.float32)
    xa[:, :D - 1] = 2.0 * xr[:, :D - 1]
    xa[:, D - 1] = 1.0
    x8 = np.clip(xa, -240, 240).astype(F8)
    # [q, d] -> [p, c, i, q] with d = c*256 + i*128 + p
    xt = np.ascontiguousarray(
        x8.T.reshape(2, 2, 128, B).transpose(2, 0, 1, 3)).reshape(128, -1)

    in_maps = []
    for core in range(NCORES):
        lo = core * NSHARD
        n_real = min(N - lo, NSHARD)
        shard = np.zeros((NSHARD, D), np.float32)
        shard[:n_real, :D - 1] = br[lo:lo + n_real, :D - 1]
        shard[:n_real, D - 1] = np.clip(512.0 - bn[lo:lo + n_real], -240.0, 240.0)
        shard[n_real:, D - 1] = -240.0
        s8 = shard.astype(F8)
        # [pt, d] -> [p, t, i, c, j] with pt = t*512 + j, d = c*256 + i*128 + p
        bt = np.ascontiguousarray(
            s8.reshape(NTILES, 512, 2, 2, 128).transpose(4, 0, 3, 2, 1)
        ).reshape(128, -1)
        in_maps.append({"xt": xt, "bt": bt})
    return xn, in_maps


def kernel(x: np.ndarray, base_data: np.ndarray, k) -> np.ndarray:
    from concourse import bass_utils

    k = int(np.asarray(k))
    assert k <= 8, f"kernel supports k<=8, got {k}"

    xn, in_maps = _prep_inputs(x, base_data)
    nc = _get_module()
    res = bass_utils.run_bass_kernel_spmd(
        nc, in_maps, core_ids=list(range(NCORES)))
    _cache["last_results"] = res

    # device out is [p, qb, cand]; query q = qb*128 + p
    s_cand = np.concatenate(
        [np.asarray(res.results[i]["out"]).astype(np.float32)
         .reshape(128, QBLK, CAND).transpose(1, 0, 2).reshape(B, CAND)
         for i in range(NCORES)], axis=1)          # [B, 8*CAND]
    d2 = xn[:, None] + 512.0 - s_cand
    dists = np.sqrt(np.maximum(d2, 0.0))
    dists.sort(axis=1)
    return np.ascontiguousarray(dists[:, :k]).astype(np.float32)
